# Initial kernel scaffold
#
"""Trainium2 kernel for nn_Attention_39204461478201.

The reference computes
    scores  = einsum('bqh,bkh->bqk', x, x) / sqrt(H)
    weights = softmax(scores, axis=1)          # over the q axis!
    context = einsum('bqk,bkh->bqh', weights, x)
    out     = mean(context, axis=1)
Because the softmax normalizes over axis=1 (q), every column of `weights`
sums to 1:  sum_q w[b,q,k] = 1 for all (b,k).  Therefore
    out[b,h] = (1/T) sum_q sum_k w[b,q,k] x[b,k,h]
             = (1/T) sum_k x[b,k,h] * (sum_q w[b,q,k])
             = mean(x, axis=1)[b,h]
— the attention collapses exactly to mean pooling over the time axis.

Device kernel: batch-parallel over 8 cores (2 batches/core).  Each core
streams its 8 MB slice from HBM and reduces it on the TensorEngine:
    psum[1,512] += w[128,1].T @ tile[128,512]     (PSUM-accumulated)
with w = 1/T = 2^-11 (a power of two, so the f32r multiply is exact).
float32r streams 1 column/cycle at N=512, so PE time (~8 us) hides
entirely under the DMA (~23 us at ~358 GB/s HBM per core).
"""

import numpy as np

B, T, H = 16, 2048, 512
N_CORES = 8
B_PER = B // N_CORES      # batches per core
P = 128                   # SBUF partitions
RB = T // P               # 16 row-blocks of [128, H] per batch
G = 4                     # row-blocks per DMA -> 1 MB per DMA, 4 DMAs/batch

_prog_cache = {}


def _build_program():
    if "nc" in _prog_cache:
        return _prog_cache["nc"]

    import concourse.bass as bass
    import concourse.tile as tile
    from concourse import bacc, mybir

    nc = bacc.Bacc(
        "TRN2", target_bir_lowering=False, debug=False, num_devices=N_CORES
    )
    x = nc.dram_tensor("x", (B_PER, T, H), mybir.dt.float32r, kind="ExternalInput")
    out = nc.dram_tensor("out", (B_PER, H), mybir.dt.float32, kind="ExternalOutput")

    NG = RB // G
    with tile.TileContext(nc) as tc:
        with (
            tc.tile_pool(name="w", bufs=1) as wpool,
            tc.tile_pool(name="xin", bufs=B_PER * NG) as xpool,
            tc.tile_pool(name="ps", bufs=B_PER, space=bass.MemorySpace.PSUM) as pspool,
            tc.tile_pool(name="res", bufs=B_PER) as respool,
        ):
            w = wpool.tile([P, 1], mybir.dt.float32r)
            nc.vector.memset(w[:], 1.0 / T)
            for b in range(B_PER):
                xb = x.ap()[b].rearrange("(r p) h -> p r h", p=P)  # [128, RB, H]
                ps = pspool.tile([1, H], mybir.dt.float32)
                for g in range(NG):
                    t = xpool.tile([P, G, H], mybir.dt.float32r)
                    nc.sync.dma_start(t[:], xb[:, g * G : (g + 1) * G, :])
                    for r in range(G):
                        nc.tensor.matmul(
                            ps[:],
                            w[:],
                            t[:, r, :],
                            start=(g == 0 and r == 0),
                            stop=(g == NG - 1 and r == G - 1),
                        )
                res = respool.tile([1, H], mybir.dt.float32)
                nc.scalar.copy(res[:], ps[:])
                nc.sync.dma_start(out.ap()[b : b + 1, :], res[:])
    nc.compile()
    _prog_cache["nc"] = nc
    return nc


def kernel(lstm_out, **_unused):
    from concourse.bass_utils import run_bass_kernel_spmd

    x = np.ascontiguousarray(np.asarray(lstm_out), dtype=np.float32)
    assert x.shape == (B, T, H), x.shape
    in_maps = [{"x": x[i * B_PER : (i + 1) * B_PER]} for i in range(N_CORES)]
    nc = _build_program()
    res = run_bass_kernel_spmd(nc, in_maps, core_ids=list(range(N_CORES)))
    return np.concatenate([r["out"] for r in res.results], axis=0)


# revision 4
# speedup vs baseline: 1.7348x; 1.7348x over previous
"""Trainium2 kernel for nn_Attention_39204461478201.

The reference computes
    scores  = einsum('bqh,bkh->bqk', x, x) / sqrt(H)
    weights = softmax(scores, axis=1)          # over the q axis!
    context = einsum('bqk,bkh->bqh', weights, x)
    out     = mean(context, axis=1)
Because the softmax normalizes over axis=1 (q), every column of `weights`
sums to 1:  sum_q w[b,q,k] = 1 for all (b,k).  Therefore
    out[b,h] = (1/T) sum_q sum_k w[b,q,k] x[b,k,h]
             = (1/T) sum_k x[b,k,h] * (sum_q w[b,q,k])
             = mean(x, axis=1)[b,h]
— the attention collapses exactly to mean pooling over the time axis.

Device kernel: batch-parallel over 8 cores (2 batches/core).  Each core
streams its 8 MB slice from HBM and reduces it on the TensorEngine:
    psum[1,512] += w[128,1].T @ tile[128,512]     (PSUM-accumulated)
with w = 1/T = 2^-11 (a power of two, so the f32r multiply is exact).
float32r streams 1 column/cycle at N=512, so PE time (~8 us) hides
entirely under the DMA (~23 us at ~358 GB/s HBM per core).
"""

import numpy as np

B, T, H = 16, 2048, 512
N_CORES = 8
B_PER = B // N_CORES      # batches per core
P = 128                   # SBUF partitions
RB = T // P               # 16 row-blocks of [128, H] per batch
G = 4                     # row-blocks per DMA -> 1 MB per DMA, 4 DMAs/batch

_prog_cache = {}


def _build_program(n_iters=1):
    if n_iters in _prog_cache:
        return _prog_cache[n_iters]

    import concourse.bass as bass
    import concourse.tile as tile
    from concourse import bacc, mybir

    nc = bacc.Bacc(
        "TRN2", target_bir_lowering=False, debug=False, num_devices=N_CORES
    )
    x = nc.dram_tensor("x", (B_PER, T, H), mybir.dt.float32r, kind="ExternalInput")
    out = nc.dram_tensor("out", (B_PER, H), mybir.dt.float32, kind="ExternalOutput")

    NG = RB // G
    with tile.TileContext(nc) as tc:
        with (
            tc.tile_pool(name="w", bufs=1) as wpool,
            tc.tile_pool(name="xin", bufs=B_PER * NG) as xpool,
            tc.tile_pool(name="ps", bufs=B_PER, space=bass.MemorySpace.PSUM) as pspool,
            tc.tile_pool(name="res", bufs=B_PER) as respool,
        ):
            w = wpool.tile([P, 1], mybir.dt.float32)
            nc.vector.memset(w[:], 1.0 / T)
            w_r = w[:].bitcast(mybir.dt.float32r)
            for _it in range(n_iters):
                for b in range(B_PER):
                    xb = x.ap()[b].rearrange("(r p) h -> p r h", p=P)  # [128, RB, H]
                    ps = pspool.tile([1, H], mybir.dt.float32)
                    for g in range(NG):
                        t = xpool.tile([P, G, H], mybir.dt.float32r)
                        nc.sync.dma_start(t[:], xb[:, g * G : (g + 1) * G, :])
                        for r in range(G):
                            nc.tensor.matmul(
                                ps[:],
                                w_r,
                                t[:, r, :],
                                start=(g == 0 and r == 0),
                                stop=(g == NG - 1 and r == G - 1),
                            )
                    res = respool.tile([1, H], mybir.dt.float32)
                    nc.scalar.copy(res[:], ps[:])
                    nc.sync.dma_start(out.ap()[b : b + 1, :], res[:])
    nc.compile()
    _prog_cache[n_iters] = nc
    return nc


def kernel(lstm_out, **_unused):
    from concourse.bass_utils import run_bass_kernel_spmd

    x = np.ascontiguousarray(np.asarray(lstm_out), dtype=np.float32)
    assert x.shape == (B, T, H), x.shape
    in_maps = [{"x": x[i * B_PER : (i + 1) * B_PER]} for i in range(N_CORES)]
    nc = _build_program()
    res = run_bass_kernel_spmd(nc, in_maps, core_ids=list(range(N_CORES)))
    return np.concatenate([r["out"] for r in res.results], axis=0)
